# revision 1
# baseline (speedup 1.0000x reference)
"""Adaptive embedding (4-cluster masked embedding + projection) on 8 trn2 cores.

Sharding: data-parallel over the batch dim — each of the 8 NeuronCores handles
one batch row (2048 tokens); the embedding/projection tables are replicated.

Host does ROUTING only (cluster assignment, stable sort, padded index arrays);
the device gathers rows from the full tables with indirect DMA, projects
clusters 1-3 on the PE (fp32), and writes cluster-sorted output rows. The host
inverse-permutes rows into token order afterwards.

The sqrt(D_PROJ)=32 output scale is an exact power of two, so it is folded
into the emb0 table and the projection matrices bit-exactly.
"""

import os

import numpy as np

CUTOFFS = (0, 20000, 40000, 200000, 267735)
D_PROJ = 1024
DES = (1024, 256, 64, 16)
N_CORES = 8
P = 128

_BUILD_CACHE = {}
LAST_RESULT = None  # BassKernelResults of the most recent run (for profiling)


def _build(caps, vocab_sizes, mm_dtype="float32r"):
    """Build the SPMD Bass program for per-cluster tile capacities `caps`
    (number of 128-token tiles per cluster, identical on every core)."""
    import concourse.bass as bass
    import concourse.bacc as bacc
    import concourse.tile as tile
    from concourse import mybir
    from concourse.masks import make_identity

    f32 = mybir.dt.float32
    fmm = getattr(mybir.dt, mm_dtype)  # float32r: single-pass fp32 matmul
    i32 = mybir.dt.int32
    nts = list(caps)
    ntsum = sum(nts)

    nc = bacc.Bacc("TRN2", target_bir_lowering=False)
    emb = [
        nc.dram_tensor(f"emb{i}", [vocab_sizes[i], DES[i]], f32, kind="ExternalInput")
        for i in range(4)
    ]
    proj = [None] + [
        nc.dram_tensor(f"proj{i}", [DES[i], D_PROJ], f32, kind="ExternalInput")
        for i in (1, 2, 3)
    ]
    # all clusters' index columns in one tensor: one DMA, earliest gather start
    idx_all = nc.dram_tensor("idx_all", [P, ntsum], i32, kind="ExternalInput")
    out = [
        nc.dram_tensor(f"out{i}", [nts[i] * P, D_PROJ], f32, kind="ExternalOutput")
        for i in range(4)
    ]

    with tile.TileContext(nc) as tc:
        with (
            tc.tile_pool(name="const", bufs=1) as cpool,
            tc.tile_pool(name="xt", bufs=6) as xtpool,
            tc.tile_pool(name="stage", bufs=8) as spool,
            tc.tile_pool(name="tpsum", bufs=2, space="PSUM") as tppool,
            tc.tile_pool(name="mpsum", bufs=3, space="PSUM") as mpool,
        ):
            idxt_all = cpool.tile([P, ntsum], i32, name="idxt_all")
            nc.sync.dma_start(out=idxt_all[:], in_=idx_all[:])
            col0 = [0, nts[0], nts[0] + nts[1], nts[0] + nts[1] + nts[2]]
            idxt = [idxt_all[:, col0[i] : col0[i] + nts[i]] for i in range(4)]

            ident = cpool.tile([P, P], f32, name="ident")
            make_identity(nc, ident)

            # Projection weights in SBUF with K on partitions. The PE consumes
            # them as float32r (single-pass fp32), which requires the SBUF
            # producer to round to f32r — stage fp32, then DVE-copy-cast.
            # proj2 first: cluster 2 is processed first.
            def load_proj_mm(name, src, rows):
                s = spool.tile([rows, D_PROJ], f32, tag="st", name=f"{name}_s")
                nc.sync.dma_start(out=s[:], in_=src)
                t = cpool.tile([rows, D_PROJ], fmm, name=name)
                nc.vector.tensor_copy(out=t[:], in_=s[:])
                return t

            p2t = load_proj_mm("p2t", proj[2][:], 64)
            p1k = [
                load_proj_mm(f"p1k{k}", proj[1][k * P : (k + 1) * P, :], P)
                for k in range(2)
            ]
            p3t = load_proj_mm("p3t", proj[3][:], 16)

            # Woven per-tile order across compute clusters: cluster 2's tiles
            # arrive gather-paced and leave PE idle gaps — spreading cluster
            # 1/3 tiles between them keeps the PE dense through the whole
            # gather phase instead of backlogging 1+3 after the gathers end.
            def weave():
                items = []
                for i in (2, 1, 3):
                    for t in range(nts[i]):
                        items.append(((t + 0.5) / nts[i], i == 2, i, t))
                items.sort(key=lambda it: (it[0], not it[1]))
                return [(i, t) for _, _, i, t in items]

            order = weave()

            # Indirect-DMA gathers. HW processes one index per partition and
            # copies out-free-size contiguous elements, so each 128-token tile
            # needs its own gather (idx column t). Cluster 0 (copy-only) last.
            g = [None] * 4
            for i in range(4):
                g[i] = cpool.tile([P, nts[i] * DES[i]], f32, name=f"g{i}")

            def gather_tile(i, ti):
                de = DES[i]
                nc.gpsimd.indirect_dma_start(
                    out=g[i][:, ti * de : (ti + 1) * de],
                    out_offset=None,
                    in_=emb[i][:],
                    in_offset=bass.IndirectOffsetOnAxis(
                        ap=idxt_all[:, col0[i] + ti : col0[i] + ti + 1], axis=0
                    ),
                )

            for i, t in order:
                gather_tile(i, t)
            for t in range(nts[0]):
                gather_tile(0, t)

            # Cluster 0 needs no projection: straight copy to DRAM.
            for t in range(nts[0]):
                nc.sync.dma_start(
                    out=out[0][t * P : (t + 1) * P, :],
                    in_=g[0][:, t * D_PROJ : (t + 1) * D_PROJ],
                )

            # Per 128-token tile: PE-transpose the gathered rows so K (=de)
            # sits on partitions (each chunk lands at partition 0 — PE needs
            # lhsT/rhs partition bases to match), project, evacuate, store.
            pws = {1: p1k, 2: [p2t], 3: [p3t]}

            def project_tile(i, t):
                de = DES[i]
                nk = (de + P - 1) // P
                pw = pws[i]
                lhs = []
                for k in range(nk):
                    w = min(P, de - k * P)
                    tp = tppool.tile([w, P], f32, tag="tp", name=f"tp{i}_{t}_{k}")
                    x = xtpool.tile([w, P], fmm, tag="xt", name=f"xt{i}_{t}_{k}")
                    lo = t * de + k * P
                    nc.tensor.transpose(
                        out=tp[:], in_=g[i][:, lo : lo + w], identity=ident[:]
                    )
                    nc.vector.tensor_copy(out=x[:], in_=tp[:])
                    lhs.append(x)
                ps = mpool.tile([P, D_PROJ], f32, tag="ps", name=f"ps{i}_{t}")
                for n in range(2):
                    for k, (lap, pwk) in enumerate(zip(lhs, pw)):
                        nc.tensor.matmul(
                            ps[:, n * 512 : (n + 1) * 512],
                            lap[:],
                            pwk[:, n * 512 : (n + 1) * 512],
                            start=(k == 0),
                            stop=(k == len(lhs) - 1),
                        )
                st = spool.tile([P, D_PROJ], f32, tag="st", name=f"st{i}_{t}")
                nc.vector.tensor_copy(out=st[:, 0:512], in_=ps[:, 0:512])
                nc.scalar.copy(out=st[:, 512:1024], in_=ps[:, 512:1024])
                nc.sync.dma_start(out=out[i][t * P : (t + 1) * P, :], in_=st[:])

            for i, t in order:
                project_tile(i, t)

    nc.compile()
    return nc


def kernel(tokens, emb0, emb1, emb2, emb3, proj1, proj2, proj3):
    global LAST_RESULT
    from concourse.bass_utils import run_bass_kernel_spmd

    toks = np.asarray(tokens).astype(np.int64, copy=False)
    nb, ns = toks.shape
    assert nb == N_CORES and ns % P == 0

    embs = [np.ascontiguousarray(np.asarray(e, dtype=np.float32)) for e in (emb0, emb1, emb2, emb3)]
    # sqrt(1024) = 32: exact power of two, folding is bit-exact.
    scale = np.float32(32.0)
    emb0s = embs[0] * scale
    projs = {
        i: np.ascontiguousarray(np.asarray(p, dtype=np.float32)) * scale
        for i, p in ((1, proj1), (2, proj2), (3, proj3))
    }

    cuts = np.asarray(CUTOFFS, dtype=np.int64)
    cluster = np.searchsorted(cuts[1:-1], toks, side="right")

    orders, counts, locs = [], [], []
    for c in range(nb):
        cl = cluster[c]
        orders.append(np.argsort(cl, kind="stable"))
        counts.append(np.bincount(cl, minlength=4))
        sizes = np.asarray([embs[i].shape[0] for i in range(4)], dtype=np.int64)
        locs.append(
            np.clip(toks[c] - cuts[cl], 0, sizes[cl] - 1).astype(np.int32)
        )
    counts = np.stack(counts)  # [nb, 4]

    caps = tuple(
        int(max(1, -(-int(counts[:, i].max()) // P))) for i in range(4)
    )  # 128-token tiles per cluster, uniform across cores
    vocab_sizes = tuple(e.shape[0] for e in embs)
    mm_dtype = os.environ.get("KERNEL_MM_DTYPE", "float32r")
    key = (caps, vocab_sizes, mm_dtype)
    if key not in _BUILD_CACHE:
        _BUILD_CACHE[key] = _build(caps, vocab_sizes, mm_dtype)
    nc = _BUILD_CACHE[key]

    in_maps = []
    for c in range(nb):
        m = {
            "emb0": emb0s,
            "emb1": embs[1],
            "emb2": embs[2],
            "emb3": embs[3],
            "proj1": projs[1],
            "proj2": projs[2],
            "proj3": projs[3],
        }
        starts = np.concatenate([[0], np.cumsum(counts[c])])
        li = locs[c][orders[c]]  # local indices, cluster-sorted
        cols = []
        for i in range(4):
            padded = np.zeros(caps[i] * P, np.int32)
            padded[: counts[c, i]] = li[starts[i] : starts[i + 1]]
            # device layout: idx[p, t] = sorted position t*128 + p
            cols.append(padded.reshape(caps[i], P).T)
        m["idx_all"] = np.ascontiguousarray(np.concatenate(cols, axis=1))
        in_maps.append(m)

    res = run_bass_kernel_spmd(nc, in_maps, core_ids=list(range(N_CORES)))
    LAST_RESULT = res

    out = np.empty((nb, ns, D_PROJ), np.float32)
    for c in range(nb):
        segs = [res.results[c][f"out{i}"][: counts[c, i]] for i in range(4)]
        out[c][orders[c]] = np.concatenate(segs, axis=0)
    return out



# revision 5
# speedup vs baseline: 1.0187x; 1.0187x over previous
"""Adaptive embedding (4-cluster masked embedding + projection) on 8 trn2 cores.

Sharding: data-parallel over the batch dim - each of the 8 NeuronCores handles
one batch row (2048 tokens); tables replicated.

Design (v3):
- Host does routing only: cluster assignment, stable sort, shard split,
  padded int16 index arrays, parity/sub-row masks.
- All gathers are single dma_gather instructions (SWDGE ucode, 16-lane
  descriptor fan-out) instead of one indirect DMA per 128-token tile: the
  ~1us fixed SWDGE cost per instruction made per-tile gathers the bottleneck.
  dma_gather's int16 indices cap a table at 32767 rows, so:
    c0 (20000x1024): direct gather (row-major), stored as-is (no projection).
    c1 (20000x256):  transpose-mode gather -> lhsT directly, no PE transpose.
    c2 (160000x64):  table repacked as 80000x128 "super-2" rows (two adjacent
                     rows per 256B row), 3 vocab shards; transpose-mode gather
                     gives K=128 columns holding [even|odd] row pairs; a
                     per-token parity mask zeroes the wrong half and the
                     projection uses W2 stacked twice on K.
    c3 (67735x16):   table repacked 8467x128 "super-8"; one transpose-mode
                     gather, 8-way sub-row mask, W3 tiled 8x on K.
- Weights are fp8e4m3 (values ~N(0,0.64) after folding the 32x output scale):
  halves weight DMA and validated to keep max rel err ~9e-3 (gate 2e-2).
- PSUM evacuation casts fp32->fp16 and alternates DVE/Activation; output
  tensors are fp16 (bf16 for c0), upcast on host.
- The PE is kept busy with dummy matmuls during the gather phase so the HAM
  clock-gate releases (1.2 -> 2.4 GHz) before the real matmuls arrive.
"""

import os

import numpy as np
import ml_dtypes

BF16 = ml_dtypes.bfloat16
FP8 = ml_dtypes.float8_e4m3

CUTOFFS = (0, 20000, 40000, 200000, 267735)
D_PROJ = 1024
N_CORES = 8
P = 128
SHARD2 = 26667  # super-2 rows per cluster-2 vocab shard (3 shards cover 80000)

_BUILD_CACHE = {}
LAST_RESULT = None


def _wrap16(idx, ncols):
    """int16 index array in dma_gather's wrapped layout: item k at [k%16, k//16],
    replicated to all 8 q7 cores (16-partition groups). Returns [128, ncols]."""
    w = np.zeros((16, ncols), np.int16)
    w[:, : len(idx) // 16] = np.asarray(idx, np.int16).reshape(-1, 16).T
    return np.tile(w, (8, 1))


def _build(caps, cap2, nwarm):
    import concourse.bass as bass
    import concourse.bacc as bacc
    import concourse.tile as tile
    from concourse import mybir

    f32 = mybir.dt.float32
    bf16 = mybir.dt.bfloat16
    f16 = mybir.dt.float16
    fp8 = mybir.dt.float8e4
    i16 = mybir.dt.int16

    nt0, nt1, nt2, nt3 = caps  # 128-token tiles per cluster (nt2 = 3*cap2/128)
    n1, n3 = nt1 * P, nt3 * P
    n2 = 3 * cap2
    assert nt2 * P == n2

    # uidx column sections (int16): c1, c3, c2 shards x3, c0
    w1c, w3c, w2c, w0c = nt1 * P // 16, n3 // 16, cap2 // 16, nt0 * P // 16
    u_off = np.cumsum([0, w1c, w3c, w2c, w2c, w2c, w0c])
    ucols = int(u_off[-1])
    mcols = n2 + n3  # masks: c2 then c3

    nc = bacc.Bacc("TRN2", target_bir_lowering=False)
    emb0 = nc.dram_tensor("emb0", [20000, 1024], bf16, kind="ExternalInput")
    emb1 = nc.dram_tensor("emb1", [20000, 256], bf16, kind="ExternalInput")
    emb2 = nc.dram_tensor("emb2", [80000, 128], bf16, kind="ExternalInput")
    emb3 = nc.dram_tensor("emb3", [8467, 128], bf16, kind="ExternalInput")
    w1d = nc.dram_tensor("w1", [256, D_PROJ], fp8, kind="ExternalInput")
    w2d = nc.dram_tensor("w2e", [128, D_PROJ], fp8, kind="ExternalInput")
    w3d = nc.dram_tensor("w3e", [128, D_PROJ], fp8, kind="ExternalInput")
    uidxd = nc.dram_tensor("uidx", [P, ucols], i16, kind="ExternalInput")
    masksd = nc.dram_tensor("masks", [P, mcols], bf16, kind="ExternalInput")
    out0 = nc.dram_tensor("out0", [nt0 * P, D_PROJ], bf16, kind="ExternalOutput")
    out1 = nc.dram_tensor("out1", [n1, D_PROJ], f16, kind="ExternalOutput")
    out2 = nc.dram_tensor("out2", [n2, D_PROJ], f16, kind="ExternalOutput")
    out3 = nc.dram_tensor("out3", [n3, D_PROJ], f16, kind="ExternalOutput")

    with tile.TileContext(nc) as tc:
        with (
            tc.tile_pool(name="const", bufs=1) as cpool,
            tc.tile_pool(name="mpsum", bufs=3, space="PSUM") as mpool,
            tc.tile_pool(name="wpsum", bufs=1, space="PSUM") as wpool,
        ):
            # --- input loads ---
            uidx = cpool.tile([P, ucols], i16, name="uidx")
            nc.sync.dma_start(out=uidx[:], in_=uidxd[:])

            w1 = [cpool.tile([P, D_PROJ], fp8, name=f"w1_{k}") for k in range(2)]
            for k in range(2):
                nc.scalar.dma_start(out=w1[k][:], in_=w1d[k * P : (k + 1) * P, :])
            masks = cpool.tile([P, mcols], bf16, name="masks")
            nc.scalar.dma_start(out=masks[:], in_=masksd[:])
            w3 = cpool.tile([P, D_PROJ], fp8, name="w3e")
            nc.scalar.dma_start(out=w3[:], in_=w3d[:])
            w2 = cpool.tile([P, D_PROJ], fp8, name="w2e")
            nc.scalar.dma_start(out=w2[:], in_=w2d[:])

            # --- PE warmup (HAM clock-gate release) while gathers run ---
            wsrc = cpool.tile([P, 512], bf16, name="wsrc")
            nc.vector.memset(wsrc[:], 0.0)
            wps = wpool.tile([P, 512], f32, name="wps")
            for _ in range(nwarm):
                nc.tensor.matmul(
                    wps[:], wsrc[:, 0:P], wsrc[:], start=True, stop=True
                )

            # --- gathers (all on gpsimd SWDGE; one instruction per source) ---
            xT1 = cpool.tile([P, 2, n1], bf16, name="xT1")  # [K%128, chunk, tok]
            nc.gpsimd.dma_gather(
                xT1[:], emb1[:], uidx[:, u_off[0] : u_off[1]], n1, n1, 256,
                transpose=True,
            )
            xT3 = cpool.tile([P, 1, n3], bf16, name="xT3")
            nc.gpsimd.dma_gather(
                xT3[:], emb3[:], uidx[:, u_off[1] : u_off[2]], n3, n3, 128,
                transpose=True,
            )
            xT2 = cpool.tile([P, 1, n2], bf16, name="xT2")
            sb = [0, SHARD2, 2 * SHARD2, 80000]
            for s in range(3):
                nc.gpsimd.dma_gather(
                    xT2[:, :, s * cap2 : (s + 1) * cap2],
                    emb2[sb[s] : sb[s + 1], :],
                    uidx[:, u_off[2 + s] : u_off[3 + s]],
                    cap2, cap2, 128,
                    transpose=True,
                )
            g0 = cpool.tile([P, nt0, 1024], bf16, name="g0")
            nc.gpsimd.dma_gather(
                g0[:], emb0[:], uidx[:, u_off[5] : u_off[6]], nt0 * P, nt0 * P,
                1024,
            )

            # --- projection pipeline ---
            xm2 = cpool.tile([P, n2], bf16, name="xm2")
            xm3 = cpool.tile([P, n3], bf16, name="xm3")
            st1 = cpool.tile([P, n1 * 8], f16, name="st1")  # n1*8 = nt1*1024
            st2 = cpool.tile([P, nt2 * D_PROJ], f16, name="st2")
            st3 = cpool.tile([P, nt3 * D_PROJ], f16, name="st3")

            ev = [0]

            def evac(dst, ps):
                e = [nc.scalar.copy, nc.vector.tensor_copy][ev[0] % 2]
                ev[0] += 1
                e(out=dst, in_=ps)

            mul = mybir.AluOpType.mult

            def project(st, t, lhs_ap, rhs_list):
                ps = mpool.tile([P, D_PROJ], f32, tag="ps", name=f"ps{ev[0]}")
                for n in range(2):
                    for k, (lap, rhs) in enumerate(rhs_list):
                        nc.tensor.matmul(
                            ps[:, n * 512 : (n + 1) * 512],
                            lap,
                            rhs[:, n * 512 : (n + 1) * 512],
                            start=(k == 0),
                            stop=(k == len(rhs_list) - 1),
                        )
                evac(st[:, t * D_PROJ : (t + 1) * D_PROJ], ps[:])

            # c1: K=256 via 2 chunks, no mask
            for t in range(nt1):
                project(
                    st1, t, None,
                    [(xT1[:, k, t * P : (t + 1) * P], w1[k]) for k in range(2)],
                )
            # c3: mask then single-chunk matmul
            for t in range(nt3):
                cols = slice(t * P, (t + 1) * P)
                nc.vector.tensor_tensor(
                    out=xm3[:, cols], in0=xT3[:, 0, cols],
                    in1=masks[:, n2 + t * P : n2 + (t + 1) * P], op=mul,
                )
                project(st3, t, None, [(xm3[:, cols], w3)])
            # c2: parity mask (late tiles masked on gpsimd, which is free
            # after its gathers) then single-chunk matmul
            for t in range(nt2):
                cols = slice(t * P, (t + 1) * P)
                eng = nc.vector if t < nt2 // 2 else nc.gpsimd
                eng.tensor_tensor(
                    out=xm2[:, cols], in0=xT2[:, 0, cols],
                    in1=masks[:, cols], op=mul,
                )
                project(st2, t, None, [(xm2[:, cols], w2)])

            # --- stores (few large DMAs on the sync sequencer) ---
            def store(dram, st, t0, t1):
                dst = dram[t0 * P : t1 * P, :].rearrange("(t p) i -> p t i", p=P)
                nc.sync.dma_start(out=dst, in_=st[:, t0 * D_PROJ : t1 * D_PROJ])

            store(out1, st1, 0, nt1)
            store(out3, st3, 0, min(3, nt3))
            if nt3 > 3:
                store(out3, st3, 3, nt3)
            q = nt2 // 4
            store(out2, st2, 0, q)
            store(out2, st2, q, 2 * q)
            nc.sync.dma_start(
                out=out0[:].rearrange("(t p) i -> p t i", p=P), in_=g0[:]
            )
            store(out2, st2, 2 * q, 3 * q)
            store(out2, st2, 3 * q, nt2)

    nc.compile()
    return nc


def kernel(tokens, emb0, emb1, emb2, emb3, proj1, proj2, proj3):
    global LAST_RESULT
    from concourse.bass_utils import run_bass_kernel_spmd

    toks = np.asarray(tokens).astype(np.int64, copy=False)
    nb, ns = toks.shape
    assert nb == N_CORES and ns % P == 0

    scale = np.float32(32.0)  # sqrt(1024): exact power of two, folded in
    emb0b = np.ascontiguousarray((np.asarray(emb0, np.float32) * scale).astype(BF16))
    emb1b = np.ascontiguousarray(np.asarray(emb1, np.float32).astype(BF16))
    emb2b = np.ascontiguousarray(
        np.asarray(emb2, np.float32).astype(BF16).reshape(80000, 128)
    )
    e3 = np.asarray(emb3, np.float32).astype(BF16)
    e3p = np.zeros((67736, 16), BF16)
    e3p[:67735] = e3
    emb3b = np.ascontiguousarray(e3p.reshape(8467, 128))
    w1 = np.ascontiguousarray((np.asarray(proj1, np.float32) * scale).astype(FP8))
    w2 = (np.asarray(proj2, np.float32) * scale).astype(FP8)
    w2e = np.ascontiguousarray(np.concatenate([w2, w2], axis=0))
    w3 = (np.asarray(proj3, np.float32) * scale).astype(FP8)
    w3e = np.ascontiguousarray(np.tile(w3, (8, 1)))

    cuts = np.asarray(CUTOFFS, dtype=np.int64)
    cluster = np.searchsorted(cuts[1:-1], toks, side="right")

    percore = []
    for c in range(nb):
        cl = cluster[c]
        li = toks[c] - cuts[cl]
        d = {}
        for i in range(4):
            m = np.where(cl == i)[0]  # stable order
            d[i] = (m, li[m])
        # c2: shard by super-row
        m2, li2 = d[2]
        sup = li2 // 2
        par = (li2 % 2).astype(np.int8)
        shard = np.minimum(sup // SHARD2, 2)
        so = np.argsort(shard, kind="stable")
        d["c2"] = (m2[so], sup[so], par[so], shard[so])
        percore.append(d)

    cnt = np.array(
        [[len(percore[c][i][0]) for i in range(4)] for c in range(nb)]
    )
    cnt2s = np.array(
        [
            [(percore[c]["c2"][3] == s).sum() for s in range(3)]
            for c in range(nb)
        ]
    )
    caps01 = [int(-(-max(1, cnt[:, i].max()) // P)) for i in (0, 1)]
    cap2 = int(-(-max(1, cnt2s.max()) // P)) * P
    nt3 = int(-(-max(1, cnt[:, 3].max()) // P))
    caps = (caps01[0], caps01[1], 3 * cap2 // P, nt3)
    nwarm = int(os.environ.get("KERNEL_NWARM", "48"))

    key = (caps, cap2, nwarm)
    if key not in _BUILD_CACHE:
        _BUILD_CACHE[key] = _build(caps, cap2, nwarm)
    nc = _BUILD_CACHE[key]

    nt0, nt1, nt2, _ = caps
    n1, n2, n3 = nt1 * P, 3 * cap2, nt3 * P
    in_maps = []
    for c in range(nb):
        d = percore[c]
        idx = []
        for arr, n in (
            (d[1][1], n1),
            (d[3][1] // 8, n3),
        ):
            a = np.zeros(n, np.int64)
            a[: len(arr)] = arr
            idx.append(_wrap16(a, n // 16))
        m2, sup, par, shard = d["c2"]
        for s in range(3):
            a = np.zeros(cap2, np.int64)
            v = sup[shard == s] - s * SHARD2
            a[: len(v)] = v
            idx.append(_wrap16(a, cap2 // 16))
        a = np.zeros(nt0 * P, np.int64)
        a[: len(d[0][1])] = d[0][1]
        idx.append(_wrap16(a, nt0 * P // 16))
        uidx = np.ascontiguousarray(np.concatenate(idx, axis=1))

        masks = np.zeros((P, n2 + n3), BF16)
        col = 0
        for s in range(3):
            p_s = par[shard == s]
            k = len(p_s)
            msk = np.zeros((P, cap2), np.float32)
            msk[:64, :k] = (p_s == 0).astype(np.float32)
            msk[64:, :k] = (p_s == 1).astype(np.float32)
            masks[:, col : col + cap2] = msk.astype(BF16)
            col += cap2
        sub3 = d[3][1] % 8
        msk3 = np.zeros((P, n3), np.float32)
        for b in range(8):
            sel = np.where(sub3 == b)[0]
            msk3[16 * b : 16 * (b + 1), sel] = 1.0
        masks[:, n2:] = msk3.astype(BF16)

        in_maps.append(
            {
                "emb0": emb0b, "emb1": emb1b, "emb2": emb2b, "emb3": emb3b,
                "w1": w1, "w2e": w2e, "w3e": w3e,
                "uidx": uidx, "masks": np.ascontiguousarray(masks),
            }
        )

    res = run_bass_kernel_spmd(nc, in_maps, core_ids=list(range(N_CORES)))
    LAST_RESULT = res

    out = np.empty((nb, ns, D_PROJ), np.float32)
    for c in range(nb):
        d = percore[c]
        r = res.results[c]
        out[c][d[0][0]] = np.asarray(
            r["out0"][: cnt[c, 0]], dtype=np.float32
        )
        out[c][d[1][0]] = np.asarray(
            r["out1"][: cnt[c, 1]], dtype=np.float32
        )
        m2 = d["c2"][0]
        shard = d["c2"][3]
        o2 = np.asarray(r["out2"], dtype=np.float32)
        pos = 0
        for s in range(3):
            k = int((shard == s).sum())
            out[c][m2[pos : pos + k]] = o2[s * cap2 : s * cap2 + k]
            pos += k
        out[c][d[3][0]] = np.asarray(r["out3"][: cnt[c, 3]], dtype=np.float32)
    return out
